# revision 29
# baseline (speedup 1.0000x reference)
"""Paged-KV GQA decode attention on 8 TRN2 NeuronCores.

Strategy (data-parallel over flattened 128-token tiles, mixed precision):
  * Host: resolve the paged cache (arange fast path -> zero-copy reshape),
    apply the store_kvcache update, slice each sequence's valid prefix,
    pad to 128-token tiles.
  * Sequences are split by context length: short seqs (ctx <= BF_THRESH)
    keep K/V in bf16 (quantization error does not average out over few
    tokens); long seqs carry K/V in fp8 e4m3 (halves the dominant DMA
    traffic; error averages to ~1e-2 max-rel, under the 2e-2 gate).
    q and w stay bf16 everywhere.
  * Per tile, per kv-head: scoresT = K_tile^T @ q (PE, stationary=K^T so
    scores land [s, h]), w = exp(SCALE*scores) (ACT), o += V^T @ w (PE),
    l = ones^T @ w (PE). Unnormalized (o, l) stream back to HBM.
  * Host: per-seq sum over tiles, subtract exp(0)=1 per zero-padded slot
    from l, divide, transpose.

DMA schedule (the kernel is DMA-stream-bound at ~26 GB/s per SDMA engine
x 16 engines):
  * The fp8 KV stream rides the sync-engine HWDGE queue alone; q and the
    bf16 KV tiles ride the scalar-engine HWDGE queue so they never delay
    the fp8 stream's first bytes.
  * Group sizes ramp 1,1,2,2,... so the PE's first weight tile arrives
    ~6us earlier than with uniform 1MB groups, and stay small (2 tiles =
    512KB) so PE-visible completion granularity is fine and the 8 HWDGE
    sem lanes recycle smoothly.
"""

import math
import os

import numpy as np

B, H, KVH, D = 32, 32, 8, 128
G = H // KVH
BLOCK_SIZE = 16
MAX_BLOCKS = 256
NUM_BLOCKS = B * MAX_BLOCKS
MAX_KV = MAX_BLOCKS * BLOCK_SIZE
SCALE = 0.08838834764831845
NCORES = 8
TILE = 128

BF_THRESH = int(os.environ.get("BASS_BF_THRESH", "768"))
OGRP = 8  # tiles per output DMA group

LAST_RESULT = None  # BassKernelResults of the most recent run (for test.py)

_NC_CACHE = {}


def _group_sizes(n_e4):
    """Ramped DMA group sizes (in tiles) covering n_e4 tiles.

    Small leading groups let the PE start early; big (8-tile = 2MB)
    steady-state groups keep the 8 HWDGE sem lanes holding ~16MB of
    queued transfers so the SDMA engines never starve; small trailing
    groups keep the last tiles from waiting on a 2MB transfer's
    completion receipt.  Returns (sizes, ramp_n) where ramp_n is the
    tile count of the leading ramp (used for compute-order placement).
    """
    lead = [1, 1, 2, 4]
    tail = [4, 2, 1, 1]
    if n_e4 >= sum(lead) + sum(tail) + 8:
        mid = n_e4 - sum(lead) - sum(tail)
        sizes = list(lead) + [8] * (mid // 8)
        if mid % 8:
            sizes.append(mid % 8)
        sizes += tail
        ramp_n = sum(lead)
    else:  # small streams: greedy doubling ramp
        sizes = []
        rem, s = n_e4, 1
        while rem:
            t = min(s, rem)
            sizes.append(t)
            rem -= t
            s = min(s * 2, 8)
        ramp_n = n_e4
    assert sum(sizes) == n_e4, (sizes, n_e4)
    return sizes, ramp_n


def _install_trace_shim():
    """Register the axon NTFF profile hook (missing from the stub antenv) and
    stub the S3 artifact upload, so trace=True yields exec_time_ns."""
    import sys
    import types

    if "antenv.axon_hooks" not in sys.modules:
        mod = types.ModuleType("antenv.axon_hooks")
        _hook = [None]
        mod.set_axon_ntff_profile_hook = lambda h: _hook.__setitem__(0, h)
        mod.get_axon_ntff_profile_hook = lambda: _hook[0]
        sys.modules["antenv.axon_hooks"] = mod
        import antenv

        antenv.axon_hooks = mod
    from antenv.axon_hooks import (
        get_axon_ntff_profile_hook,
        set_axon_ntff_profile_hook,
    )

    if get_axon_ntff_profile_hook() is None:
        try:
            from trn_agent_boot.trn_boot import _ntff_profile_via_ctypes

            set_axon_ntff_profile_hook(
                _ntff_profile_via_ctypes("/opt/axon/libaxon_pjrt.so")
            )
        except Exception:
            pass
    import concourse.bass_utils as bu

    bu.upload_artifacts = lambda tmpdir: f"file://{tmpdir}"


def _build_nc(n_bf: int, n_e4: int):
    """One SPMD program: n_e4 fp8 tiles then n_bf bf16 tiles per core."""
    import concourse.mybir as mybir
    import concourse.tile as tile
    from concourse import bacc

    key = (n_bf, n_e4)
    if key in _NC_CACHE:
        return _NC_CACHE[key]

    BF = mybir.dt.bfloat16
    E4 = mybir.dt.float8e4
    F32 = mybir.dt.float32
    Exp = mybir.ActivationFunctionType.Exp

    n_t = n_bf + n_e4
    n_og = (n_t + OGRP - 1) // OGRP
    sizes, ramp_n = _group_sizes(n_e4)
    starts = [sum(sizes[:i]) for i in range(len(sizes))]

    KCOLS = KVH * TILE  # 1024
    TCOLS = 2 * KCOLS   # 2048 cols per tile (K^T then V)

    nc = bacc.Bacc("TRN2", target_bir_lowering=False, num_devices=NCORES)
    xb = nc.dram_tensor("xb", [TILE, max(n_bf, 1) * TCOLS], BF,
                        kind="ExternalInput")
    x4 = nc.dram_tensor("x4", [TILE, max(n_e4, 1) * TCOLS], E4,
                        kind="ExternalInput")
    qd = nc.dram_tensor("qd", [TILE, n_t * H], BF, kind="ExternalInput")
    yo = nc.dram_tensor("yo", [n_og, TILE, OGRP * H], BF, kind="ExternalOutput")
    yl = nc.dram_tensor("yl", [n_og, 1, OGRP * H], F32, kind="ExternalOutput")

    with tile.TileContext(nc) as tc:
        with (
            tc.tile_pool(name="consts", bufs=1) as consts,
            tc.tile_pool(name="kvb", bufs=1) as kvb_pool,
            tc.tile_pool(name="kv4", bufs=1) as kv4_pool,
            tc.tile_pool(name="wt", bufs=4) as wt_pool,
            tc.tile_pool(name="outs", bufs=6) as out_pool,
            tc.tile_pool(name="ps_sc", bufs=4, space="PSUM") as ps_sc,
            tc.tile_pool(name="ps_o", bufs=3, space="PSUM") as ps_o,
            tc.tile_pool(name="ps_l", bufs=1, space="PSUM") as ps_l,
        ):
            # q head: the first few tiles' q columns ride ahead of the KV
            # stream on the sync queue (16KB -> lands + completes fast)
            qhead = min(16, n_t) * H
            q_sb = consts.tile([TILE, n_t * H], BF)
            nc.sync.dma_start(out=q_sb[:, :qhead], in_=qd[:, :qhead])
            # fp8 KV stream: all group DMAs queued upfront on the sync
            # HWDGE queue, ramped sizes, so HWDGE streams back-to-back and
            # the first weight tile lands ASAP.
            kv_grps = []
            for g, (a, s) in enumerate(zip(starts, sizes)):
                kv_g = kv4_pool.tile([TILE, s * TCOLS], E4, tag=f"g{g}",
                                     name=f"kv_g{g}")
                nc.sync.dma_start(out=kv_g,
                                  in_=x4[:, a * TCOLS:(a + s) * TCOLS])
                kv_grps.append(kv_g)
            # q tail + bf16 KV ride the scalar HWDGE queue (parallel)
            if qhead < n_t * H:
                nc.scalar.dma_start(out=q_sb[:, qhead:], in_=qd[:, qhead:])
            if n_bf > 0:
                kv_bf = kvb_pool.tile([TILE, n_bf * TCOLS], BF)
                nc.scalar.dma_start(out=kv_bf, in_=xb[:, :])
            ones = consts.tile([TILE, 1], BF)
            nc.vector.memset(ones, 1.0)

            # compute position p -> (sbuf tile, col base).  Compute order:
            # ramp + 8 e4 tiles, then bf16 tiles (by then their data has
            # arrived on the scalar queue), then the remaining e4 stream.
            bp = min(ramp_n + 8, n_e4)
            def tile_src(p):
                if bp <= p < bp + n_bf:
                    return kv_bf, (p - bp) * TCOLS
                idx = p if p < bp else p - n_bf
                g = 0
                while not (starts[g] <= idx < starts[g] + sizes[g]):
                    g += 1
                return kv_grps[g], (idx - starts[g]) * TCOLS

            state = {"yo": None, "yl": None, "w": None}

            def consume(t, kv_t, base, w_g, wo, wcols):
                """PV + l + output staging for tile t (runs one tile behind
                the QK stream so the in-order PE queue never waits on exp)."""
                o_ps = ps_o.tile([TILE, H], F32)
                for kh in range(KVH):
                    nc.tensor.matmul(
                        o_ps[:, kh * G:(kh + 1) * G],
                        lhsT=kv_t[:, base + KCOLS + kh * TILE:
                                  base + KCOLS + (kh + 1) * TILE],
                        rhs=w_g[:, wo + kh * G:wo + (kh + 1) * G],
                        start=(kh == 0),
                        stop=(kh == KVH - 1),
                    )
                if t % OGRP == 0:
                    state["yo"] = out_pool.tile([TILE, OGRP * H], BF, tag="yo",
                                                name="yo_sb")
                    state["yl"] = out_pool.tile([1, OGRP * H], F32, tag="yl",
                                                name="yl_sb")
                off = (t % OGRP) * H
                nc.vector.tensor_copy(state["yo"][:, off:off + H], o_ps)
                # one l matmul per completed w group
                if wo + H == wcols or t == n_t - 1:
                    n = wo + H
                    l_ps = ps_l.tile([1, n], F32)
                    nc.tensor.matmul(l_ps, lhsT=ones, rhs=w_g[:, :n],
                                     start=True, stop=True)
                    lo = (t % OGRP) - (n // H - 1)
                    nc.vector.tensor_copy(
                        state["yl"][0:1, lo * H:lo * H + n], l_ps)
                if t % OGRP == OGRP - 1 or t == n_t - 1:
                    nc.gpsimd.dma_start(out=yo[t // OGRP], in_=state["yo"])
                    nc.gpsimd.dma_start(out=yl[t // OGRP], in_=state["yl"])

            # w tiles are grouped in blocks of 4 tiles, aligned to OGRP so
            # the l matmul columns stay within one output group
            WGRP = 4
            pending = []
            for t in range(n_t):
                kv_t, base = tile_src(t)
                # scoresT = K^T @ q : [s=128, H]
                sc = ps_sc.tile([TILE, H], F32)
                for kh in range(KVH):
                    nc.tensor.matmul(
                        sc[:, kh * G:(kh + 1) * G],
                        lhsT=kv_t[:, base + kh * TILE:base + (kh + 1) * TILE],
                        rhs=q_sb[:, t * H + kh * G:t * H + (kh + 1) * G],
                        start=(kh == 0),
                        stop=(kh == KVH - 1),
                    )
                if t % WGRP == 0:
                    ncols = min(WGRP, n_t - t) * H
                    state["w"] = wt_pool.tile([TILE, ncols], BF, tag="w",
                                              name="w_sb")
                    state["wcols"] = ncols
                wo = (t % WGRP) * H
                nc.scalar.activation(state["w"][:, wo:wo + H],
                                     sc, Exp, bias=0.0, scale=SCALE)
                # run PV two tiles behind QK so the PE never waits on the
                # exp ACT (trace showed PV matmuls waiting ~0.2-0.4us on
                # the activation sem at one-behind depth)
                if len(pending) == 2:
                    consume(*pending.pop(0))
                pending.append((t, kv_t, base, state["w"], wo, state["wcols"]))
            for p in pending:
                consume(*p)
    nc.finalize()
    _NC_CACHE[key] = nc
    return nc


def kernel(q, k, v, k_cache, v_cache, block_tables, context_lens, slot_mapping):
    global LAST_RESULT
    import ml_dtypes
    from concourse.bass_utils import run_bass_kernel_spmd

    trace = bool(os.environ.get("BASS_TRACE"))
    if trace:
        _install_trace_shim()

    BF = ml_dtypes.bfloat16
    E4 = ml_dtypes.float8_e4m3

    q = np.asarray(q, dtype=np.float32)
    k = np.asarray(k, dtype=np.float32)
    v = np.asarray(v, dtype=np.float32)
    k_cache = np.asarray(k_cache)
    v_cache = np.asarray(v_cache)
    block_tables = np.asarray(block_tables)
    context_lens = np.asarray(context_lens).astype(np.int64)
    slot_mapping = np.asarray(slot_mapping).astype(np.int64)

    # --- resolve paged layout -------------------------------------------------
    if np.array_equal(block_tables.ravel(), np.arange(NUM_BLOCKS, dtype=np.int64)):
        k_seq = k_cache.reshape(B, MAX_KV, KVH, D)  # zero-copy view
        v_seq = v_cache.reshape(B, MAX_KV, KVH, D)
        flat_pos = slot_mapping  # slot index == b*MAX_KV + pos under arange tables
    else:  # general fallback: true gather (slow, but correct for any table)
        k_seq = k_cache[block_tables].reshape(B, MAX_KV, KVH, D)
        v_seq = v_cache[block_tables].reshape(B, MAX_KV, KVH, D)
        blk = slot_mapping // BLOCK_SIZE
        off = slot_mapping % BLOCK_SIZE
        flat_pos = np.empty(B, np.int64)
        for b in range(B):
            tb = np.where(block_tables[b] == blk[b])[0][0]
            flat_pos[b] = b * MAX_KV + tb * BLOCK_SIZE + off[b]

    # --- tile map: class (bf16 short / fp8 long), global order per class -----
    ctx = context_lens.astype(np.int64)
    n_t_seq = [int(math.ceil(int(c) / TILE)) for c in ctx]
    is_bf = [int(c) <= BF_THRESH for c in ctx]
    order_bf = [b for b in range(B) if is_bf[b]]
    order_e4 = [b for b in range(B) if not is_bf[b]]
    g_bf = sum(n_t_seq[b] for b in order_bf)
    g_e4 = sum(n_t_seq[b] for b in order_e4)
    n_bf = (g_bf + NCORES - 1) // NCORES
    n_e4 = (g_e4 + NCORES - 1) // NCORES
    n_t = n_bf + n_e4
    _, ramp_n = _group_sizes(n_e4)
    bp = min(ramp_n + 8, n_e4)

    def gt_of(idx, bf):
        """Compute-order position of class-local tile idx (matches device)."""
        if bf:
            return bp + idx
        return idx if idx < bp else idx + n_bf

    # class-tile-start per seq (within its class's global stream)
    start_of = {}
    acc = 0
    for b in order_bf:
        start_of[b] = acc
        acc += n_t_seq[b]
    acc = 0
    for b in order_e4:
        start_of[b] = acc
        acc += n_t_seq[b]

    # --- pack host arrays -----------------------------------------------------
    KCOLS = KVH * TILE
    TCOLS = 2 * KCOLS
    xb = np.zeros((NCORES, TILE, max(n_bf, 1) * TCOLS), BF)
    x4 = np.zeros((NCORES, TILE, max(n_e4, 1) * TCOLS), E4)
    qd = np.zeros((NCORES, TILE, n_t * H), BF)

    for b in range(B):
        c = int(ctx[b])
        nt = n_t_seq[b]
        kb = np.zeros((nt * TILE, KVH, D), np.float32)
        vb = np.zeros((nt * TILE, KVH, D), np.float32)
        kb[:c] = k_seq[b, :c]
        vb[:c] = v_seq[b, :c]
        # store_kvcache: new token for seq b lands at flat_pos[b] % MAX_KV
        p = int(flat_pos[b] - b * MAX_KV)
        if 0 <= p < c:
            kb[p] = k[b]
            vb[p] = v[b]
        # K^T tiles [t, d, kh, s]; V tiles [t, s, kh*d]
        kt = kb.reshape(nt, TILE, KVH, D).transpose(0, 3, 2, 1).reshape(
            nt, D, KVH * TILE)
        vt = vb.reshape(nt, TILE, KVH * D)
        dt_np = BF if is_bf[b] else E4
        kv = np.concatenate([kt, vt], axis=2).astype(dt_np)  # [nt, 128, 2048]
        qT = q[b].T.astype(BF)  # [d, H]
        n_cl = n_bf if is_bf[b] else n_e4
        a = start_of[b]
        for j in range(nt):
            core, idx = (a + j) // n_cl, (a + j) % n_cl
            if is_bf[b]:
                xb[core, :, idx * TCOLS:(idx + 1) * TCOLS] = kv[j]
            else:
                x4[core, :, idx * TCOLS:(idx + 1) * TCOLS] = kv[j]
            gt = gt_of(idx, is_bf[b])
            qd[core, :, gt * H:(gt + 1) * H] = qT

    in_maps = [
        {"xb": xb[c0], "x4": x4[c0], "qd": qd[c0]} for c0 in range(NCORES)
    ]

    nc = _build_nc(n_bf, n_e4)
    res = run_bass_kernel_spmd(
        nc, in_maps, core_ids=list(range(NCORES)), trace=trace
    )
    LAST_RESULT = res

    # --- host reduction -------------------------------------------------------
    # per core: yo [n_og, 128, OGRP*H] bf16, yl [n_og, 1, OGRP*H] f32
    yo_all = [np.asarray(res.results[c]["yo"], dtype=np.float32) for c in range(NCORES)]
    yl_all = [np.asarray(res.results[c]["yl"], dtype=np.float32) for c in range(NCORES)]

    out = np.empty((B, H, D), np.float32)
    for b in range(B):
        c = int(ctx[b])
        nt = n_t_seq[b]
        n_cl = n_bf if is_bf[b] else n_e4
        a = start_of[b]
        o_b = np.zeros((D, H), np.float32)
        l_b = np.zeros(H, np.float32)
        for j in range(nt):
            core, idx = (a + j) // n_cl, (a + j) % n_cl
            gt = gt_of(idx, is_bf[b])
            o_b += yo_all[core][gt // OGRP][:, (gt % OGRP) * H:(gt % OGRP + 1) * H]
            l_b += yl_all[core][gt // OGRP][0, (gt % OGRP) * H:(gt % OGRP + 1) * H]
        l_b = l_b - (nt * TILE - c)  # remove exp(0) pad terms
        out[b] = (o_b / l_b).T
    return out
